# revision 120
# baseline (speedup 1.0000x reference)
"""GQA attention kernel for Trainium2, sharded over 8 NeuronCores.

Problem: B=2, S=2048, D=2048, H=16 query heads, KV=4 kv heads, HD=128,
RoPE, no causal mask, out = softmax(q k^T / sqrt(HD)) v @ Wo.

Sharding: core = b*4 + g  (b in {0,1} batch, g in {0..3} head group).
Each core handles 4 query heads [4g..4g+3] and kv head g (exact GQA
split), with Wq/Wk/Wv column-sliced and Wo row-sliced.  Each core
produces a partial o_proj output for its batch; host sums the 4 partials
per batch.

Per-core layout strategy (all matmuls bf16 with fp32 PSUM accumulation):
  - host supplies h[b]^T pre-tiled (contraction dim D on partitions,
    contiguous 4KB DMA runs per s-tile)
  - phase 1: q/k/v projections -> [s, cols] tiles (k|v fused in one
    N=256 matmul); RoPE in fp32 with head-broadcast APs (6 DVE ops);
    q_rot/k_rot PE-transposed to qT/kT [HD, S].  The last attention
    block's q RoPE+transposes are deferred past the phase boundary so
    kT (which gates attention) completes as early as possible
  - phases 2+3 run as ONE flat software pipeline over (block, head,
    pair): scores pair j -> PSUM [128,1024]; exp on ScalarE (with the
    1/sqrt(HD) scale folded in, free); PV lags scores by TWO pairs so
    its exp is always long finished; the PREVIOUS block's o_proj
    matmuls are sprinkled between pairs as PE filler (phase 2 alone is
    ACT-exp bound; the quads soak up PE slack so neither engine idles)
  - softmax denominators NEVER touch PE: DVE accumulates each exp pair
    into a running bf16 acc (2x DVE perf mode; the bf16 partial-sum
    rounding is ~0.2% rms, well in budget), Pool's partition_all_reduce
    finishes the cross-key reduction, DVE reciprocal + multiply
    normalize.  This removed 256 512-col PE matmuls (~55us) vs the
    matmul-denominator design
  - exp straight out of PSUM, no max subtraction (scores ~N(0,1))
  - o_proj "quads": 4 accumulating matmuls (ch order ends at the
    latest-finishing head) -> [128,512] PSUM in ps_c -> DVE/ACT copy ->
    DMA; quad slots skip the first iterations after a block boundary
    (previous block's last normalize hasn't landed in aoT yet)
  - deferred q RoPE runs on the otherwise-idle Pool engine, ONE op per
    iteration (a burst would delay the all-reduces queued behind it),
    transposes one head later (after the ps_b slot's divide frees)
  - PSUM (8 banks): wide 2x2-bank (score pairs / ph1 ps_q), B 2x1
    (ph1 transposes / ph2 ps_o), C 2x1 (ph1 kv / o_proj quads)
  - fp8 DoubleRow scores were tried and REVERTED: concentrated-softmax
    queries amplify the ~4% fp8 q/k quantization into ~6% output error.
    Pool/GpSimd cannot read PSUM on real hw (BIR verifier rejects it),
    and DVE has no divide op — both discovered via the hw compile.

Cost-model timeline (TimelineSim): 274.6 us/core (was 329.6 baseline),
PE ~92% occupied at 253.5us busy vs a 250.1us bf16 MAC floor; residual
gaps are DMA-serialized startup (~5.8us), end-of-kernel drain (~3.9us)
and block-boundary normalize chains.  Validated on the axon hw path:
rel(max) 0.0062 vs the 2e-2 gate.
"""

import math
import numpy as np
import ml_dtypes

B, S, D = 2, 2048, 2048
H, KV, HD = 16, 4, 128
G = 4          # tensor-parallel head groups
HG = H // G    # 4 query heads per core
QCOLS = HG * HD  # 512
P = 128
NT = S // P    # 16 sequence tiles
KO = D // P    # 16 contraction chunks
NB = S // 512  # 4 query blocks of 512

BF16 = ml_dtypes.bfloat16

_CACHE = {}


def _build_nc():
    import concourse.mybir as mybir
    import concourse.tile as tile
    from concourse import bacc
    from concourse.masks import make_identity
    from contextlib import ExitStack

    dt = mybir.dt
    nc = bacc.Bacc(
        "TRN2",
        target_bir_lowering=False,
        debug=False,
        enable_asserts=False,
        num_devices=8,
    )

    # hT pre-tiled on host: hT4[i, p, ko, sc] = h.T[ko*128+p, i*128+sc]
    # so each DMA'd s-tile is one contiguous [128, KO*128] block (4KB runs)
    hT = nc.dram_tensor(
        "hT", [S // 128, 128, (D // 128) * 128], dt.bfloat16, kind="ExternalInput"
    ).ap()
    wq = nc.dram_tensor("wq", [D, QCOLS], dt.bfloat16, kind="ExternalInput").ap()
    wk = nc.dram_tensor("wk", [D, HD], dt.bfloat16, kind="ExternalInput").ap()
    wv = nc.dram_tensor("wv", [D, HD], dt.bfloat16, kind="ExternalInput").ap()
    wo = nc.dram_tensor("wo", [QCOLS, D], dt.bfloat16, kind="ExternalInput").ap()
    cosd = nc.dram_tensor("cosd", [S, HD], dt.float32, kind="ExternalInput").ap()
    sind = nc.dram_tensor("sind", [S, HD], dt.float32, kind="ExternalInput").ap()
    # y ships as bf16: halves the output DMA bytes and shortens the
    # end-of-kernel copy->DMA->semaphore drain; host sums partials in f32
    y = nc.dram_tensor("y", [S, D], dt.bfloat16, kind="ExternalOutput").ap()

    with tile.TileContext(nc) as tc:
        _emit(tc, nc, mybir, hT, wq, wk, wv, wo, cosd, sind, y, make_identity)

    nc.compile()
    return nc


def _emit(tc, nc, mybir, hT, wq, wk, wv, wo, cosd, sind, y, make_identity):
    import os
    from contextlib import ExitStack
    from concourse import bass_isa

    PHASES = os.environ.get("K_PHASES", "123")

    dt = mybir.dt
    bf16 = dt.bfloat16
    f32 = dt.float32
    Exp = mybir.ActivationFunctionType.Exp

    with ExitStack() as ctx:
        const = ctx.enter_context(tc.tile_pool(name="const", bufs=1))
        wpool = ctx.enter_context(tc.tile_pool(name="wpool", bufs=1))
        big = ctx.enter_context(tc.tile_pool(name="big", bufs=1))
        hpool = ctx.enter_context(tc.tile_pool(name="hpool", bufs=5))
        work = ctx.enter_context(tc.tile_pool(name="work", bufs=4))
        expp = ctx.enter_context(tc.tile_pool(name="expp", bufs=6))
        # PSUM: "wide" = 2-bank slots for paired score tiles (also ph1 ps_q);
        # B = transposes (ph1) / ps_o (ph2); C = kv (ph1) / o_proj quads
        # (ph2/3, own pool so quads never wait on an exp-held slot).
        # 2*2 + 2 + 2 = 8 banks.
        ps_wide = ctx.enter_context(tc.tile_pool(name="ps_wide", bufs=2, space="PSUM"))
        ps_b = ctx.enter_context(tc.tile_pool(name="ps_b", bufs=2, space="PSUM"))
        ps_c = ctx.enter_context(tc.tile_pool(name="ps_c", bufs=2, space="PSUM"))

        # --- constants ---
        ident = const.tile([P, P], bf16)
        make_identity(nc, ident)

        # --- hT prefetch helper (pre-tiled on host: hT[i] = [128, KO*128]) --
        ht_tiles = {}

        def load_ht(i, split=False):
            if i not in ht_tiles:
                hT_t = hpool.tile([P, KO, P], bf16, tag="ht", name=f"ht{i}")
                src = hT[i].rearrange("p (ko s) -> p ko s", ko=KO)
                if split:
                    # first tile: 4 smaller DMAs so the first projection
                    # matmuls start as soon as their ko chunks land
                    for g in range(0, KO, 4):
                        nc.sync.dma_start(
                            hT_t[:, g : g + 4], src[:, g : g + 4]
                        )
                else:
                    nc.sync.dma_start(hT_t, src)
                ht_tiles[i] = hT_t
            return ht_tiles[i]

        # --- weights and tables to SBUF ---
        # DMA emission order drives the model's serial DMA queue: first two
        # hT tiles and the first weight chunks go first so the projection
        # matmuls can start immediately; wo (phase 3) goes last.
        wq_sb = wpool.tile([P, KO, QCOLS], bf16)
        wkv_sb = wpool.tile([P, KO, 2 * HD], bf16)
        cos_sb = wpool.tile([P, NT, HD], f32)
        sin_sb = wpool.tile([P, NT, HD], f32)
        wq_r = wq.rearrange("(ko p) m -> p ko m", p=P)
        wk_r = wk.rearrange("(ko p) m -> p ko m", p=P)
        wv_r = wv.rearrange("(ko p) m -> p ko m", p=P)
        cos_r = cosd.rearrange("(i p) c -> p i c", p=P)
        sin_r = sind.rearrange("(i p) c -> p i c", p=P)
        KG = 4  # ko chunks per DMA
        # startup-critical order: the DMA device transfers strictly serially,
        # and tile 0's q matmuls (the long pole) need hT0 + wq g0.  Ship the
        # first half of hT0, then wq g0 (q matmuls for ko 0-7 can start),
        # then the rest; kv weights follow (tile 0 runs q-first, kv after)
        if "1" in PHASES:
            hT_t0 = hpool.tile([P, KO, P], bf16, tag="ht", name="ht0")
            src0 = hT[0].rearrange("p (ko s) -> p ko s", ko=KO)
            nc.sync.dma_start(hT_t0[:, 0:8], src0[:, 0:8])
            nc.sync.dma_start(wq_sb[:, 0:KG], wq_r[:, 0:KG])
            nc.sync.dma_start(hT_t0[:, 8:KO], src0[:, 8:KO])
            ht_tiles[0] = hT_t0
        else:
            nc.sync.dma_start(wq_sb[:, 0:KG], wq_r[:, 0:KG])
        # wq g1 before the kv weights: tile 0's q matmuls (ko 4-7) are the
        # next thing PE needs after g0
        nc.sync.dma_start(wq_sb[:, KG : 2 * KG], wq_r[:, KG : 2 * KG])
        nc.sync.dma_start(wkv_sb[:, 0:KG, :HD], wk_r[:, 0:KG])
        nc.sync.dma_start(wkv_sb[:, 0:KG, HD:], wv_r[:, 0:KG])
        for kg in range(2 * KG, KO, KG):
            ks = slice(kg, kg + KG)
            nc.sync.dma_start(wq_sb[:, ks], wq_r[:, ks])
        if "1" in PHASES:
            load_ht(1)
        for kg in range(KG, KO, KG):
            ks = slice(kg, kg + KG)
            nc.sync.dma_start(wkv_sb[:, ks, :HD], wk_r[:, ks])
            nc.sync.dma_start(wkv_sb[:, ks, HD:], wv_r[:, ks])
        if "1" in PHASES:
            load_ht(2)
            load_ht(3)
        for kg in range(0, KO, KG):
            ts_ = slice(kg, kg + KG)  # 4 s-tiles of rope tables per chunk
            nc.sync.dma_start(cos_sb[:, ts_], cos_r[:, ts_])
            nc.sync.dma_start(sin_sb[:, ts_], sin_r[:, ts_])

        # --- persistent intermediates ---
        # qT and kT fused: [hd, 5, s] with slots 0..3 = q heads, slot 4 = k.
        # (fp8 scores were tried and revert: concentrated-softmax queries
        # amplify the ~4% fp8 q/k quantization into ~6% output error.)
        qkT = big.tile([P, HG + 1, S], bf16)
        qT = qkT[:, :HG]                   # [hd, head, s]
        kT = qkT[:, HG]                    # [hd, s]
        v_sb = big.tile([P, NT, HD], bf16)  # [s_inner, s_chunk, hd]
        aoT = big.tile([P, HG, S], bf16)   # attn_out^T  [c_inner, head, s]

        # ---------------- Phase 1: QKV projections + RoPE + transposes ------
        late_qrot = []
        for i in range(NT if "1" in PHASES else 0):
            hT_t = load_ht(i)
            if i + 3 < NT:
                load_ht(i + 3)

            ps_q = ps_wide.tile([P, 1024], f32, tag="wide", name="ps_q")[:, :512]
            ps_kv = ps_c.tile([P, 2 * HD], f32, tag="c", name="ps_kv")
            if i == 0:
                # tile 0: the full q run first (hT0 first-half + wq land
                # before the kv weights), then kv
                for ko in range(KO):
                    nc.tensor.matmul(
                        ps_q, hT_t[:, ko], wq_sb[:, ko],
                        start=(ko == 0), stop=(ko == KO - 1),
                    )
                for ko in range(KO):
                    nc.tensor.matmul(
                        ps_kv, hT_t[:, ko], wkv_sb[:, ko],
                        start=(ko == 0), stop=(ko == KO - 1),
                    )
            else:
                for ko in range(KO):
                    first, last = ko == 0, ko == KO - 1
                    nc.tensor.matmul(
                        ps_q, hT_t[:, ko], wq_sb[:, ko], start=first, stop=last
                    )
                    nc.tensor.matmul(
                        ps_kv, hT_t[:, ko], wkv_sb[:, ko], start=first, stop=last
                    )

            # v: straight cast copy into [s, hd] layout; route the last
            # tiles' copies to DVE so ACT is free when attention starts
            cp = nc.vector if i >= NT - 3 else nc.scalar
            if cp is nc.vector:
                nc.vector.tensor_copy(v_sb[:, i], ps_kv[:, HD:])
            else:
                nc.scalar.copy(v_sb[:, i], ps_kv[:, HD:])

            # q and k side by side in one [P, 5, HD] fp32 tile for fused RoPE
            qk_f = work.tile([P, HG + 1, HD], f32, tag="qkf")
            if cp is nc.vector:
                nc.vector.tensor_copy(
                    qk_f[:, :HG], ps_q.rearrange("p (h c) -> p h c", h=HG)
                )
                nc.vector.tensor_copy(qk_f[:, HG], ps_kv[:, :HD])
            else:
                nc.scalar.copy(
                    qk_f[:, :HG], ps_q.rearrange("p (h c) -> p h c", h=HG)
                )
                nc.scalar.copy(qk_f[:, HG], ps_kv[:, :HD])

            HF = HD // 2

            def rope_thunks(src, lo_h, n_h, i=i, eng=None):
                # RoPE(src[:, lo_h:lo_h+n_h]) -> (rot tile, list of op
                # thunks).  Callers either run all thunks at once (phase 1)
                # or dribble them across iterations (deferred phase-2 path,
                # so a slow Pool-engine rope never blocks the all-reduces
                # queued behind it)
                e = eng or nc.vector
                cos_t = cos_sb[:, i]
                sin_t = sin_sb[:, i]
                cos_lo = cos_t[:, None, :HF].to_broadcast((P, n_h, HF))
                cos_hi = cos_t[:, None, HF:].to_broadcast((P, n_h, HF))
                sin_lo = sin_t[:, None, :HF].to_broadcast((P, n_h, HF))
                sin_hi = sin_t[:, None, HF:].to_broadcast((P, n_h, HF))
                s = src[:, lo_h : lo_h + n_h]
                s_lo = s[:, :, :HF]
                s_hi = s[:, :, HF:]
                rot = work.tile(
                    [P, HG + 1, HD], bf16, tag="qkrot", name="rot"
                )[:, :n_h]
                t1 = work.tile([P, HG + 1, HF], f32, tag="rt1", name="t1")[:, :n_h]
                t2 = work.tile([P, HG + 1, HF], f32, tag="rt2", name="t2")[:, :n_h]
                t3 = work.tile([P, HG + 1, HF], f32, tag="rt3", name="t3")[:, :n_h]
                t4 = work.tile([P, HG + 1, HF], f32, tag="rt4", name="t4")[:, :n_h]
                thunks = [
                    lambda: e.tensor_mul(t1, s_lo, cos_lo),
                    lambda: e.tensor_mul(t2, s_hi, sin_lo),
                    lambda: e.tensor_sub(rot[:, :, :HF], t1, t2),
                    lambda: e.tensor_mul(t3, s_hi, cos_hi),
                    lambda: e.tensor_mul(t4, s_lo, sin_hi),
                    lambda: e.tensor_add(rot[:, :, HF:], t3, t4),
                ]
                return rot, thunks

            def do_rope(src, lo_h, n_h, i=i, eng=None):
                rot, thunks = rope_thunks(src, lo_h, n_h, i=i, eng=eng)
                for t in thunks:
                    t()
                return rot

            def emit_kt(k_rot, i=i):
                ps_tk = ps_b.tile([P, P], bf16, tag="b", name="ps_tk")
                nc.tensor.transpose(ps_tk, k_rot[:, 0], ident)
                nc.vector.tensor_copy(kT[:, i * P : (i + 1) * P], ps_tk)

            def emit_qt(q_rot, i=i, pool=None, ptag=None):
                # phase 1: ps_b ring (alternating with ps_tk).  Deferred
                # phase-2 calls pass ps_c: the ps_b ring is ps_o's there,
                # and sharing it would chain these transposes behind the
                # previous head's divide
                ps_t = (pool or ps_b).tile(
                    [P, HG * P], bf16, tag=ptag or "b", name="ps_t"
                )
                for h in range(HG):
                    nc.tensor.transpose(
                        ps_t[:, h * P : (h + 1) * P], q_rot[:, h], ident
                    )
                nc.vector.tensor_copy(
                    qT[:, :, i * P : (i + 1) * P],
                    ps_t.rearrange("p (h s) -> p h s", h=HG),
                )

            if i < 4 * (NB - 1):
                # fused RoPE over q heads + k, then all 10 half-transposes
                qk_rot = do_rope(qk_f, 0, HG + 1)
                emit_kt(qk_rot[:, HG : HG + 1])
                emit_qt(qk_rot)
            else:
                # last block: narrow k-only RoPE first (kT gates ALL of
                # phase 2); q RoPE + transposes deferred past the boundary
                k_rot = do_rope(qk_f, HG, 1)
                emit_kt(k_rot)
                late_qrot.append((i, qk_f, rope_thunks, emit_qt))


        # wo is only needed for o_proj: load it while phase 2 runs
        wo_sb = wpool.tile([P, HG, D], bf16)
        nc.sync.dma_start(wo_sb, wo.rearrange("(ch p) n -> p ch n", p=P))

        # ------- Phase 2 (attention) + Phase 3 (o_proj), one flat pipeline ---
        # Cross-head software pipeline: PV always lags scores by one pair,
        # ACROSS head and block boundaries, so the per-head exp-latency
        # bubble disappears.  The previous block's o_proj matmuls are
        # sprinkled between pairs as PE filler (phase 2 alone is ACT-exp
        # bound; o_proj quads soak up the PE slack so ACT never idles).
        y_r = y.rearrange("(i p) n -> p i n", p=P)

        inv_sqrt_hd = 1.0 / math.sqrt(HD)

        def emit_scores(b, h):
            qs = slice(b * 512, (b + 1) * 512)

            def go(j):
                ps_s2 = ps_wide.tile([P, 1024], f32, tag="wide", name="ps_s2")
                for r in range(2):
                    c = 2 * j + r
                    nc.tensor.matmul(
                        ps_s2[:, r * 512 : (r + 1) * 512],
                        kT[:, c * P : (c + 1) * P],
                        qT[:, h, qs],
                        start=True,
                        stop=True,
                    )
                expT = expp.tile([P, 1024], bf16, tag="exp", name="expT")
                # 1/sqrt(HD) softmax scale folded into the exp (free on ACT)
                nc.scalar.activation(expT, ps_s2, Exp, scale=inv_sqrt_hd)
                return expT

            return go

        def make_quads(b):
            # o_proj for block b as 16 PE quads (one 512-col y block each);
            # ch order 0..3 so the last head's aoT is only needed by the
            # 4th matmul of each quad.  ps_y lives in ps_c (free after ph1).
            quads = []
            for i in range(4 * b, 4 * b + 4):
                for nb2 in range(NB):
                    def quad(i=i, nb2=nb2, split_tail=False):
                        ps_y = ps_c.tile([P, 512], f32, tag="c", name="ps_y")
                        ns = slice(nb2 * 512, (nb2 + 1) * 512)
                        for ch in range(HG):
                            nc.tensor.matmul(
                                ps_y,
                                aoT[:, ch, i * P : (i + 1) * P],
                                wo_sb[:, ch, ns],
                                start=(ch == 0),
                                stop=(ch == HG - 1),
                            )
                        y_sb = work.tile([P, 512], bf16, tag="ysb", bufs=6)
                        # NOTE: Pool/GpSimd cannot read PSUM on real hw
                        # (BIR verifier rejects it) — copies stay DVE/ACT
                        if split_tail:
                            # final quad: halves on DVE+ACT in parallel and
                            # two DMAs, shortening the end-of-kernel
                            # copy->DMA->sem drain chain
                            nc.vector.tensor_copy(y_sb[:, :256], ps_y[:, :256])
                            nc.scalar.copy(y_sb[:, 256:], ps_y[:, 256:])
                            ns_a = slice(nb2 * 512, nb2 * 512 + 256)
                            ns_b = slice(nb2 * 512 + 256, (nb2 + 1) * 512)
                            nc.sync.dma_start(y_r[:, i, ns_a], y_sb[:, :256])
                            nc.sync.dma_start(y_r[:, i, ns_b], y_sb[:, 256:])
                            return
                        if nb2 % 2 == 0:
                            nc.vector.tensor_copy(y_sb, ps_y)
                        else:
                            nc.scalar.copy(y_sb, ps_y)
                        nc.sync.dma_start(y_r[:, i, ns], y_sb)
                    quads.append(quad)
            return quads

        if "2" in PHASES:
            NJ = NT // 2
            pend = []          # o_proj quads from the previous block
            pvq = []           # (ps_o, expT, j, fin) PVs lagging 2 pairs
            lateq = list(late_qrot)
            transq = []        # lateq rots awaiting their PE transposes
            dribble = []       # pending Pool rope ops, one per iteration

            def emit_pv(ps_o, expT, j):
                first, last = j == 0, j == NJ - 1
                for r in range(2):
                    c = 2 * j + r
                    sl = slice(r * 512, (r + 1) * 512)
                    nc.tensor.matmul(
                        ps_o, v_sb[:, c], expT[:, sl],
                        start=(first and r == 0), stop=(last and r == 1),
                    )

            def emit_rsum(acc):
                # denominator all-reduce + reciprocal depend only on acc,
                # not the PVs: emitted right after the head's last
                # exp-accumulate so Pool/DVE run them during the PV lag
                # window (hw DVE has no divide op, so recip+mul)
                rsum = work.tile([P, 512], f32, tag="rbc")
                nc.gpsimd.partition_all_reduce(
                    rsum, acc, channels=P, reduce_op=bass_isa.ReduceOp.add
                )
                recip_bc = work.tile([P, 512], f32, tag="recip")
                nc.vector.reciprocal(recip_bc, rsum)
                return recip_bc

            def finish_head(ps_o, recip_bc, qs, h):
                # (the mul is data-bound on its PV's completion; scheduler
                # priority hints were tried and have no effect here)
                nc.vector.tensor_mul(aoT[:, h, qs], ps_o, recip_bc)

            def pop_pv():
                ps_o_p, expT_p, j_p, fin = pvq.pop(0)
                emit_pv(ps_o_p, expT_p, j_p)
                if fin is not None:
                    # this was the head's last PV: normalize right behind it
                    finish_head(*fin)

            for b in range(NB):
                for h in range(HG):
                    qs = slice(b * 512, (b + 1) * 512)
                    scores = emit_scores(b, h)
                    ps_o = ps_b.tile([P, 512], f32, tag="b", name="ps_o")
                    acc = work.tile([P, 512], bf16, tag="acc", name="acc")
                    for j in range(NJ):
                        # PE order per iter: scores(j) first (its psum slot
                        # freed two pairs ago), then a PV lagging TWO pairs
                        # (so its exp is long finished), then quad filler
                        expT = scores(j)
                        pvq.append([ps_o, expT, j, None])
                        if len(pvq) > 2:
                            pop_pv()
                        if j % 2 == 1 and pend:
                            # defer the first slots after a block boundary:
                            # the previous block's last-head normalize
                            # hasn't landed in aoT until ~(h0, j4); catch
                            # up with double pops at (h0,j7) and (h1,j1)
                            if h == 0:
                                n = {5: 1, 7: 2}.get(j, 0)
                            elif h == 1 and j == 1:
                                n = 2
                            else:
                                n = 1
                            for _ in range(n):
                                if pend:
                                    pend.pop(0)()
                        # bf16 adds hit the DVE 2x mode; bf16 partial-sum
                        # rounding (~0.2% rms on denominators) is in budget
                        if j == 0:
                            nc.vector.tensor_add(
                                acc, expT[:, :512], expT[:, 512:]
                            )
                        else:
                            nc.vector.tensor_add(acc, acc, expT[:, :512])
                            nc.vector.tensor_add(acc, acc, expT[:, 512:])
                        if j == NJ - 1:
                            pvq[-1][3] = (ps_o, emit_rsum(acc), qs, h)
                        if j == 2 and lateq and h % 2 == 0:
                            # deferred q RoPE (block-3 queries), spread over
                            # blocks 0-1 on the otherwise-idle Pool engine,
                            # ONE op per iteration so the rope never delays
                            # the all-reduces queued behind it on Pool; the
                            # PE transposes wait until the next head's j3
                            # (rope finished AND the ps_b slot's divide done)
                            i_l, qk_f_l, thunk_fn, emit_qt_l = lateq.pop(0)
                            q_rot_l, ths = thunk_fn(
                                qk_f_l, 0, HG, eng=nc.gpsimd
                            )
                            dribble.extend(ths)
                            transq.append((q_rot_l, emit_qt_l))
                        if dribble:
                            dribble.pop(0)()
                        if j == 5 and transq and h % 2 == 1:
                            q_rot_l, emit_qt_l = transq.pop(0)
                            emit_qt_l(q_rot_l)
                if "3" in PHASES:
                    assert not pend
                    pend = make_quads(b)
            while pvq:
                pop_pv()
            for q in pend:
                q()


def get_nc():
    if "nc" not in _CACHE:
        _CACHE["nc"] = _build_nc()
    return _CACHE["nc"]


def make_in_maps(inputs):
    """Shard full inputs into 8 per-core input maps."""
    h = np.asarray(inputs["hidden_states"], dtype=np.float32)
    cos = np.asarray(inputs["cos"], dtype=np.float32).reshape(S, HD)
    sin = np.asarray(inputs["sin"], dtype=np.float32).reshape(S, HD)
    # 1/sqrt(HD) softmax scale is applied in the exp activation on-device,
    # keeping q/k at N(0,1) scale for the fp8 score path
    Wq = np.asarray(inputs["Wq"], dtype=np.float32)
    Wk = np.asarray(inputs["Wk"], dtype=np.float32)
    Wv = np.asarray(inputs["Wv"], dtype=np.float32)
    Wo = np.asarray(inputs["Wo"], dtype=np.float32)

    # hT4[i, p, ko*128+sc] = h[b].T[ko*128+p, i*128+sc]  (see dram decl)
    hT = [
        np.ascontiguousarray(
            h[b].T.reshape(KO, P, NT, P).transpose(2, 1, 0, 3).reshape(NT, P, KO * P)
        ).astype(BF16)
        for b in range(B)
    ]
    wq_s = [np.ascontiguousarray(Wq[:, g * QCOLS : (g + 1) * QCOLS]).astype(BF16) for g in range(G)]
    wk_s = [np.ascontiguousarray(Wk[:, g * HD : (g + 1) * HD]).astype(BF16) for g in range(G)]
    wv_s = [np.ascontiguousarray(Wv[:, g * HD : (g + 1) * HD]).astype(BF16) for g in range(G)]
    wo_s = [np.ascontiguousarray(Wo[g * QCOLS : (g + 1) * QCOLS, :]).astype(BF16) for g in range(G)]

    in_maps = []
    for core in range(8):
        b, g = divmod(core, G)
        in_maps.append(
            {
                "hT": hT[b],
                "wq": wq_s[g],
                "wk": wk_s[g],
                "wv": wv_s[g],
                "wo": wo_s[g],
                "cosd": cos,
                "sind": sin,
            }
        )
    return in_maps


def kernel(**inputs) -> np.ndarray:
    from concourse import bass_utils

    nc = get_nc()
    in_maps = make_in_maps(inputs)
    res = bass_utils.run_bass_kernel_spmd(nc, in_maps, core_ids=list(range(8)))
    out = np.zeros((B, S, D), dtype=np.float32)
    for core in range(8):
        b = core // G
        out[b] += np.asarray(res.results[core]["y"], dtype=np.float32)
    return out

